# revision 1
# baseline (speedup 1.0000x reference)
"""ArcFace loss kernel for Trainium2, class-sharded across 8 NeuronCores.

Swapped-orientation fp8 architecture (v2):
  - Classes live on PSUM partitions: per class-block cb (128 classes),
    pm[c, b] = sum_d w8[d, c] * xs8[d, b] with w8 = fp8(16*w) stationary and
    xs8 = fp8(16*xn) moving, using fp8 DoubleRow matmuls (2 k-pairs).
  - Per-class norms via a Gram trick: gram[c, c'] = w8_blk^T w8_blk (same
    stationary, fp8 DoubleRow); diag extracted with an identity-mask multiply
    (DVE) + free-axis reduce (GPSIMD/Pool), landing n2 in COMPACT layout
    [128, NBLK] so rsqrt (exp(-0.5*ln)) is two tiny Act ops per super-pair.
  - exp(64*cos - 16): per-class scale rwv = 4*rsqrt(n2') applied either
    directly in the Act op (scale is per-partition = per-class now!) or via a
    DVE broadcast multiply + batched Act exp, split to balance engines.
  - Row sums (over classes = partitions) via ones-matmul accumulating into a
    single PSUM bank across all class blocks.
  - Host: f64 combine of the 8 partial sums, ArcFace margin fixup on the 512
    target entries, mean cross-entropy.  Fixed shift of -16 (fits bf16 es).

Measured (8 NeuronCores, trn2): relative error vs reference 1.5e-04;
per-core HW time ~107.2us repeat-loop-measured (~98.7us after subtracting the
measured 8.5us For_i back-edge), vs 156.3us for the previous bf16
batch-on-partitions baseline (1.46x).  Loop-invariant input DMAs/constants
are hoisted out of the repeat loop and the norm tiles are split per
super-pair: without this, iteration i+1's first writes WAR-serialize against
readers at the very tail of iteration i, blocking cross-iteration overlap
(-8us measured).  Per-class scale is applied on half the
blocks via DVE broadcast-multiply + batched exp and half directly in the Act
op (dve_pairs=2 splits each 4-pair super 50/50), with 3 pm PSUM pair-buffers
(3*2 + gram 1 + sums 1 = 8 banks) for PE runahead -- each measured best on
hardware among: dve_pairs {0,2,4}, pm granularity/bufs, gram bufs {1,2},
SUP {8,16}, bf16 vs fp8e5 es.  Hardware consistently ran ~1.15-1.3x above
the TimelineSim cost model with weak config correlation, so every structural
choice here was A/B-benched on the NeuronCores, not trusted from the model.
"""

import math

import ml_dtypes
import numpy as np

# Problem constants (hardcoded per contract; kernel.py must be self-contained).
B = 512  # batch
D = 512  # feature dim
C = 100000  # classes
S = 64.0
MARGIN = 0.5
COS_M = math.cos(MARGIN)
SIN_M = math.sin(MARGIN)
TH = math.cos(math.pi - MARGIN)
MM = math.sin(math.pi - MARGIN) * MARGIN

NCORES = 8
CBLK = 128  # classes per block (PSUM partition dim)
NBLK = 100  # blocks per core
CSH = CBLK * NBLK  # 12800 classes per core
CPAD = CSH * NCORES  # 102400
KB = D // 128  # 4 contraction blocks
SUP = 8  # class blocks per super (gram/DMA batch)
DSHIFT = 16.0  # fixed logsumexp shift; es = exp(64*cos - 16) fits bf16
ES_SHIFT = 11.3  # shift for fp8e5 es: top ~3e4 < 57344, flush below cos~0
FSCALE = 16.0  # fp8 pre-scale on both x and w

_CACHE = {}


def _fix_act_tables():
    """Make Exp and Ln resolve to the one ACT table set containing both
    (avoids ~1.3us table reloads between Ln and Exp)."""
    import concourse.hw_specs as hw_specs

    tables = hw_specs.get_activation_tables("gen3")
    for name in ("exp_and_others", "natural_log"):
        if name in tables and "natural_log_exp_and_others" in tables:
            tables[name].clear()


def _supers():
    out = []
    b0 = 0
    while b0 < NBLK:
        n = min(SUP, NBLK - b0)
        out.append(list(range(b0, b0 + n)))
        b0 += n
    return out


def _build_nc(repeat=1, n_direct=0, dve_pairs=2, es_f8=True, mask_pool=False):
    import concourse.bass as bass
    import concourse.tile as tile
    from concourse import bacc, mybir

    _fix_act_tables()
    nc = bacc.Bacc(
        "TRN2",
        target_bir_lowering=False,
        debug=False,
        enable_asserts=False,
        num_devices=NCORES,
    )
    f8 = mybir.dt.float8e4
    f8e5 = mybir.dt.float8e5
    bf16 = mybir.dt.bfloat16
    f32 = mybir.dt.float32
    es_dt = f8e5 if es_f8 else bf16
    DR = mybir.MatmulPerfMode.DoubleRow

    xs8_d = nc.dram_tensor("xs8", [128, KB, B], f8, kind="ExternalInput").ap()
    w8_d = nc.dram_tensor("w8", [128, NBLK, KB, CBLK], f8, kind="ExternalInput").ap()
    idm_d = nc.dram_tensor("idm", [128, CBLK], bf16, kind="ExternalInput").ap()
    s_out = nc.dram_tensor("s_out", [B], f32, kind="ExternalOutput").ap()

    from contextlib import ExitStack, nullcontext

    supers = _supers()
    NSUP = len(supers)
    # supers routed Act-direct (per-partition scale fused into the exp op)
    if n_direct > 0:
        step = max(1, NSUP // n_direct)
        direct = set(range(1, NSUP, step))
        while len(direct) > n_direct:
            direct.pop()
    else:
        direct = set()

    with tile.TileContext(nc) as tc, ExitStack() as ctx:
        singles = ctx.enter_context(tc.tile_pool(name="singles", bufs=1))
        wsp = ctx.enter_context(tc.tile_pool(name="wsp", bufs=4))
        mskp = ctx.enter_context(tc.tile_pool(name="mskp", bufs=2))
        ysp = ctx.enter_context(tc.tile_pool(name="ysp", bufs=3))
        esp = ctx.enter_context(tc.tile_pool(name="esp", bufs=3))
        lnp = ctx.enter_context(tc.tile_pool(name="lnp", bufs=2))
        gramp = ctx.enter_context(tc.tile_pool(name="gramp", bufs=1, space="PSUM"))
        pmp = ctx.enter_context(tc.tile_pool(name="pmp", bufs=3, space="PSUM"))
        sump = ctx.enter_context(tc.tile_pool(name="sump", bufs=1, space="PSUM"))

        hint = (
            mybir.EngineType.PE,
            mybir.EngineType.Activation,
            mybir.EngineType.DVE,
            mybir.EngineType.Pool,
            mybir.EngineType.SP,
        )
        # loop-invariant inputs/constants live OUTSIDE the repeat loop so the
        # next iteration's first writes don't WAR-serialize against the tail
        # of the previous iteration
        xs8 = singles.tile([128, KB, B], f8)
        nc.sync.dma_start(out=xs8[:], in_=xs8_d)
        idm = singles.tile([128, CBLK], bf16)
        nc.sync.dma_start(out=idm[:], in_=idm_d)
        ones_bf = singles.tile([128, CBLK], bf16)
        nc.vector.memset(ones_bf[:], 1.0)
        eps_b = singles.tile([128, 1], f32)
        nc.vector.memset(eps_b[:], 1e-6)
        bias_ln4 = singles.tile([128, 1], f32)
        nc.vector.memset(bias_ln4[:], math.log(4.0))
        bias_m16 = singles.tile([128, 1], f32)
        nc.vector.memset(bias_m16[:], -(ES_SHIFT if es_f8 else DSHIFT))
        ones8 = singles.tile([128, 2, CBLK], f8e5)
        nc.vector.memset(ones8[:], 1.0)

        # per-super-pair norm tiles: iteration i+1's norm writes to pair p
        # only wait on iteration i's M(2p)/M(2p+1) readers (mid-pipeline),
        # not the whole previous iteration
        NPAIR = 7
        n2ps = [
            singles.tile([128, 2 * SUP], f32, name=f"n2p{p}") for p in range(NPAIR)
        ]
        rwps = [
            singles.tile([128, 2 * SUP], f32, name=f"rwp{p}") for p in range(NPAIR)
        ]

        ctx.enter_context(
            tc.For_i(0, repeat, 1, hint_engines=hint) if repeat > 1 else nullcontext()
        )

        sums = sump.tile([128, B], f32, tag="sums", name="sums")

        wsups = [None] * NSUP
        if es_f8:
            n_mms = sum((len(sup) + 1) // 2 for sup in supers)
        else:
            n_mms = sum(len(sup) for sup in supers)
        mm_idx = [0]  # running count for start/stop of the sum accum group

        def phase_dma(s):
            sup = supers[s]
            ns = len(sup)
            ws = wsp.tile(
                [128, SUP, KB, CBLK], f8, tag="ws", name=f"ws{s}",
                padded_shape=[128, SUP, KB, CBLK],
            )
            wsups[s] = ws
            nc.sync.dma_start(out=ws[:, :ns, :, :], in_=w8_d[:, sup[0] : sup[0] + ns, :, :])

        def phase_g(s):
            sup = supers[s]
            ns = len(sup)
            ws = wsups[s]
            # grams in half-super groups of 4 -> 1 PSUM bank each, bufs=2
            for h0 in range(0, ns, 4):
                nh = min(4, ns - h0)
                gram = gramp.tile([128, 4, CBLK], f32, tag="gram", name=f"gram{s}_{h0}")
                for bi in range(nh):
                    for kp in range(2):
                        nc.tensor.matmul(
                            gram[:, bi, :],
                            lhsT=ws[:, h0 + bi, 2 * kp : 2 * kp + 2, :],
                            rhs=ws[:, h0 + bi, 2 * kp : 2 * kp + 2, :],
                            start=(kp == 0),
                            stop=(kp == 1),
                            perf_mode=DR,
                        )
                masked = mskp.tile([128, 4, CBLK], bf16, tag="msk", name=f"msk{s}_{h0}")
                idm_b = bass.AP(
                    tensor=idm.tensor,
                    offset=idm.offset,
                    ap=[idm.ap[0], [0, nh], idm.ap[1]],
                )
                nc.vector.tensor_mul(masked[:, :nh, :], gram[:, :nh, :], idm_b)
                lo = (s % 2) * SUP + h0
                nc.vector.tensor_reduce(
                    out=n2ps[s // 2][:, lo : lo + nh],
                    in_=masked[:, :nh, :],
                    axis=mybir.AxisListType.X,
                    op=mybir.AluOpType.add,
                )

        def phase_n(s_list):
            p = s_list[0] // 2
            nb = sum(len(supers[s]) for s in s_list)
            n2l = lnp.tile([128, 2 * SUP], f32, tag="n2l", name=f"n2l{s_list[0]}")
            nc.scalar.activation(
                n2l[:, :nb],
                n2ps[p][:, :nb],
                mybir.ActivationFunctionType.Ln,
                bias=eps_b[:],
                scale=1.0,
            )
            nc.scalar.activation(
                rwps[p][:, :nb],
                n2l[:, :nb],
                mybir.ActivationFunctionType.Exp,
                bias=bias_ln4[:],
                scale=-0.5,
            )

        def phase_m(s):
            sup = supers[s]
            ns = len(sup)
            ws = wsups[s]
            es = esp.tile([128, SUP, B], es_dt, tag="es", name=f"es{s}")
            ysup = None
            if s not in direct and dve_pairs > 0:
                ysup = ysp.tile([128, SUP, B], bf16, tag="y", name=f"y{s}")
            npairs = (ns + 1) // 2
            for pi in range(npairs):
                bis = [b for b in (2 * pi, 2 * pi + 1) if b < ns]
                np_ = len(bis)
                pm = pmp.tile([128, 2, B], f32, tag="pm", name=f"pm{s}_{pi}")
                for j, bi in enumerate(bis):
                    for kp in range(2):
                        nc.tensor.matmul(
                            pm[:, j, :],
                            lhsT=ws[:, bi, 2 * kp : 2 * kp + 2, :],
                            rhs=xs8[:, 2 * kp : 2 * kp + 2, :],
                            start=(kp == 0),
                            stop=(kp == 1),
                            perf_mode=DR,
                        )
                blk0 = sup[0] + 2 * pi
                if s in direct or pi >= dve_pairs:
                    for j, bi in enumerate(bis):
                        lo = (s % 2) * SUP + 2 * pi + j
                        nc.scalar.activation(
                            es[:, bi, :],
                            pm[:, j, :],
                            mybir.ActivationFunctionType.Exp,
                            bias=bias_m16[:],
                            scale=rwps[s // 2][:, lo : lo + 1],
                        )
                else:
                    rwt = rwps[s // 2]
                    lo = (s % 2) * SUP + 2 * pi
                    rw_b = bass.AP(
                        tensor=rwt.tensor,
                        offset=rwt.offset + lo,
                        ap=[rwt.ap[0], [1, np_], [0, B]],
                    )
                    nc.vector.tensor_mul(ysup[:, 2 * pi : 2 * pi + np_, :], pm[:, :np_, :], rw_b)
                    if pi == min(dve_pairs, npairs) - 1:
                        nc.scalar.activation(
                            es[:, : 2 * pi + np_, :],
                            ysup[:, : 2 * pi + np_, :],
                            mybir.ActivationFunctionType.Exp,
                            bias=bias_m16[:],
                            scale=1.0,
                        )
            es_tiles[s] = es

        def phase_sum(s):
            es = es_tiles[s]
            ns = len(supers[s])
            if es_f8:
                for pi in range((ns + 1) // 2):
                    i = mm_idx[0]
                    mm_idx[0] += 1
                    nc.tensor.matmul(
                        sums[:, :],
                        lhsT=ones8[:],
                        rhs=es[:, 2 * pi : 2 * pi + 2, :],
                        start=(i == 0),
                        stop=(i == n_mms - 1),
                        perf_mode=DR,
                        skip_group_check=True,
                    )
            else:
                for bi in range(ns):
                    i = mm_idx[0]
                    mm_idx[0] += 1
                    nc.tensor.matmul(
                        sums[:, :],
                        lhsT=ones_bf[:],
                        rhs=es[:, bi, :],
                        start=(i == 0),
                        stop=(i == n_mms - 1),
                        skip_group_check=True,
                    )

        es_tiles = [None] * NSUP
        # software-pipelined emission: grams run 2 supers ahead of mains,
        # sum-matmuls trail mains by 1 super so PE never waits on exp(s).
        # Ready work (mains/sums) is emitted BEFORE speculative work (grams)
        # each iteration to avoid head-of-line blocking in the seq queues.
        pend_n = []
        phase_dma(0)
        for s in range(NSUP):
            if s + 1 < NSUP:
                phase_dma(s + 1)
            if s >= 2:
                phase_m(s - 2)
            if s >= 3:
                phase_sum(s - 3)
            phase_g(s)
            pend_n.append(s)
            if len(pend_n) == 2 or s == NSUP - 1:
                phase_n(pend_n)
                pend_n = []
        phase_m(NSUP - 2)
        phase_sum(NSUP - 3)
        phase_m(NSUP - 1)
        phase_sum(NSUP - 2)
        phase_sum(NSUP - 1)

        s_sb = singles.tile([1, B], f32)
        nc.vector.tensor_copy(s_sb[:], sums[0:1, :])
        nc.sync.dma_start(
            out=s_out.rearrange("(one b) -> one b", one=1), in_=s_sb[:]
        )

    nc.compile()
    return nc


def _get_nc():
    if "nc" not in _CACHE:
        _CACHE["nc"] = _build_nc()
    return _CACHE["nc"]


def _prep_inputs(x, weights):
    """Host-side shard/layout prep: normalize x, shard+scale+cast to fp8."""
    x = np.asarray(x, dtype=np.float32)
    w = np.asarray(weights, dtype=np.float32)
    f8 = ml_dtypes.float8_e4m3

    xn = x / np.linalg.norm(x.astype(np.float64), axis=1, keepdims=True)
    # xs8[p, k, b] = 16*xn[b, k*128+p]
    xs8 = np.ascontiguousarray(
        (FSCALE * xn.T.astype(np.float32)).reshape(KB, 128, B).transpose(1, 0, 2)
    ).astype(f8)

    wpad = np.zeros((CPAD, D), dtype=np.float32)
    wpad[:C] = w
    w8_maps = []
    for i in range(NCORES):
        shard = FSCALE * wpad[i * CSH : (i + 1) * CSH]  # [12800, 512]
        # w8[p, cb, k, c] = 16*shard[cb*128+c, k*128+p]
        arr = shard.reshape(NBLK, CBLK, KB, 128).transpose(3, 0, 2, 1)
        w8_maps.append(np.ascontiguousarray(arr).astype(f8))

    idm = np.eye(128, dtype=ml_dtypes.bfloat16)
    return xs8, w8_maps, idm


def _in_maps(x, weights):
    xs8, w8_maps, idm = _prep_inputs(x, weights)
    return [{"xs8": xs8, "w8": w8_maps[i], "idm": idm} for i in range(NCORES)]


def _run_on_device(in_maps, trace=False):
    from concourse.bass_utils import run_bass_kernel_spmd

    nc = _get_nc()
    res = run_bass_kernel_spmd(
        nc, in_maps, core_ids=list(range(NCORES)), trace=trace
    )
    _CACHE["last_results"] = res
    return [r["s_out"].astype(np.float64) for r in res.results]


ACTIVE_SHIFT = ES_SHIFT  # must match the es_f8 default of _build_nc


def kernel(x, weights, targets, _trace=False):
    x = np.asarray(x)
    weights = np.asarray(weights)
    targets = np.asarray(targets).astype(np.int64)

    s_shards = _run_on_device(_in_maps(x, weights), trace=_trace)

    # ---- host combine (f64, ~0.5 MFLOP total) ----
    s_total = np.sum(s_shards, axis=0)  # [B]
    # remove zero-pad classes: each contributes exp(0 - shift) exactly
    npad = CPAD - C
    s_total = s_total - npad * math.exp(-ACTIVE_SHIFT)

    xf = x.astype(np.float64)
    xn = xf / np.linalg.norm(xf, axis=1, keepdims=True)
    wtg = weights.astype(np.float64)[targets]  # [B, D] gathered target rows
    wtg = wtg / np.linalg.norm(wtg, axis=1, keepdims=True)
    cos_t = np.einsum("bd,bd->b", xn, wtg)

    sin_t = np.sqrt(np.clip(1.0 - cos_t * cos_t, 0.0, 1.0))
    phi = cos_t * COS_M - sin_t * SIN_M
    psi = np.where(cos_t > TH, phi, cos_t - MM)

    # swap the target term: remove exp(S*cos_t), add exp(S*psi)
    s_adj = s_total - np.exp(S * cos_t - ACTIVE_SHIFT) + np.exp(S * psi - ACTIVE_SHIFT)
    lse = ACTIVE_SHIFT + np.log(s_adj)
    loss = np.mean(lse - S * psi)
    return np.float32(loss)



# revision 2
# speedup vs baseline: 1.5100x; 1.5100x over previous
"""ArcFace loss kernel for Trainium2, class-sharded across 8 NeuronCores.

v3 architecture (host-normalized weights + split exp):
  - Host normalizes BOTH x rows and weight rows before fp8 quantization, so
    no per-class norm correction is needed on device at all: the entire Gram/
    rsqrt machinery of v2 is gone.  pm[c, b] = w8^T xs8 = 256*cos + quant
    noise, computed with fp8e4m3 DoubleRow matmuls (classes on PSUM
    partitions, 128-class blocks, 100 blocks/core).
  - es = exp(S*cos - SH) with a FIXED scalar scale/bias, produced two ways to
    split the elementwise load across engines:
      * Act pairs: scalar.activation Exp, scale=S/256, bias=-SH, f8e5 out.
      * DVE pairs: Schraudolph bit-trick exp - a single fused tensor_scalar
        (mult A1, add B1) converting f32->uint8 with round-to-nearest-even +
        saturation (verified on HW); the u8 result IS the f8e5m2 bit pattern
        (A1 = (S/256)*4/ln2, B1 = 60 - SH*4/ln2 + sigma).  Saturation at 0
        == +0.0 in f8e5 gives free flush-to-zero for the low tail; the high
        side would need cos > 0.53 (~12 sigma) to hit NaN encodings.
  - Row sums over classes via fp8e5 DoubleRow ones-matmuls into one PSUM
    bank (the DVE pairs' u8 tiles are bitcast back to f8e5).
  - Host: f64 combine of 8 partial sums, exact pad correction (Act-path pad
    classes contribute e^-SH each, DVE-path pads contribute exactly 0 since
    round(B1) <= 0), ArcFace margin fixup on the 512 targets, mean CE.
  - The loss is log(sum(exp)) + linear terms, so the 2e-2 rel-err gate
    allows ~e^0.9 slack on the sum; measured sim error of this scheme is
    ~2e-5..6e-4 for any Act/DVE split (sigma=-0.11 centers the bias).

Baseline v2 (Gram norms + per-class scale, measured 112.3us/iter) spent
~51us Act + ~46us DVE + ~32us PE per the cost model; v3 cuts the modeled
busy to ~27us Act + ~27us DVE + ~27us PE.
"""

import math

import ml_dtypes
import numpy as np

# Problem constants (hardcoded per contract; kernel.py must be self-contained).
B = 512  # batch
D = 512  # feature dim
C = 100000  # classes
S = 64.0
MARGIN = 0.5
COS_M = math.cos(MARGIN)
SIN_M = math.sin(MARGIN)
TH = math.cos(math.pi - MARGIN)
MM = math.sin(math.pi - MARGIN) * MARGIN

NCORES = 8
CBLK = 128  # classes per block (PSUM partition dim)
NBLK = 100  # blocks per core
CSH = CBLK * NBLK  # 12800 classes per core
CPAD = CSH * NCORES  # 102400
KB = D // 128  # 4 contraction blocks
SUP = 8  # class blocks per super (DMA batch)
FSCALE = 16.0  # fp8 pre-scale on both x and w -> pm = 256*cos

SH = 11.3  # logsumexp shift: es = exp(64*cos - SH) fits f8e5
A8 = 4.0 / math.log(2.0)  # f8e5m2 bits per ln-unit
A1 = (S / 256.0) * A8  # pm -> bits scale (Schraudolph)
SIGMA = -0.11  # mantissa-interp bias centering (sim-tuned)
B1 = 60.0 - SH * A8 + SIGMA  # bits offset; <0 so pad classes (pm=0) -> +0.0
FRAC_DVE = 0.46  # fraction of class-block pairs on the DVE path

_CACHE = {}


def _fix_act_tables():
    """Make Exp resolve to one ACT table set (avoids table reloads)."""
    import concourse.hw_specs as hw_specs

    tables = hw_specs.get_activation_tables("gen3")
    for name in ("exp_and_others", "natural_log"):
        if name in tables and "natural_log_exp_and_others" in tables:
            tables[name].clear()


def _supers():
    out = []
    b0 = 0
    while b0 < NBLK:
        n = min(SUP, NBLK - b0)
        out.append(list(range(b0, b0 + n)))
        b0 += n
    return out


def _pair_routes(frac_dve=FRAC_DVE):
    """Per class-block-pair engine routing: True -> DVE, False -> Act.
    Weighted round-robin so both engines stay busy throughout."""
    npairs = (NBLK + 1) // 2
    routes = []
    acc = 0.0
    for _ in range(npairs):
        acc += frac_dve
        if acc >= 1.0 - 1e-9:
            routes.append(True)
            acc -= 1.0
        else:
            routes.append(False)
    return routes


def _build_nc(repeat=1, frac_dve=FRAC_DVE):
    import concourse.bass as bass
    import concourse.tile as tile
    from concourse import bacc, mybir

    _fix_act_tables()
    nc = bacc.Bacc(
        "TRN2",
        target_bir_lowering=False,
        debug=False,
        enable_asserts=False,
        num_devices=NCORES,
    )
    f8 = mybir.dt.float8e4
    f8e5 = mybir.dt.float8e5
    u8 = mybir.dt.uint8
    f32 = mybir.dt.float32
    DR = mybir.MatmulPerfMode.DoubleRow

    xs8_d = nc.dram_tensor("xs8", [128, KB, B], f8, kind="ExternalInput").ap()
    w8_d = nc.dram_tensor("w8", [128, NBLK, KB, CBLK], f8, kind="ExternalInput").ap()
    s_out = nc.dram_tensor("s_out", [B], f32, kind="ExternalOutput").ap()

    from contextlib import ExitStack, nullcontext

    supers = _supers()
    NSUP = len(supers)
    routes = _pair_routes(frac_dve)

    with tile.TileContext(nc) as tc, ExitStack() as ctx:
        singles = ctx.enter_context(tc.tile_pool(name="singles", bufs=1))
        wsp = ctx.enter_context(tc.tile_pool(name="wsp", bufs=4))
        esp = ctx.enter_context(tc.tile_pool(name="esp", bufs=3))
        pmp = ctx.enter_context(tc.tile_pool(name="pmp", bufs=3, space="PSUM"))
        sump = ctx.enter_context(tc.tile_pool(name="sump", bufs=1, space="PSUM"))

        hint = (
            mybir.EngineType.PE,
            mybir.EngineType.Activation,
            mybir.EngineType.DVE,
            mybir.EngineType.Pool,
            mybir.EngineType.SP,
        )
        # loop-invariant inputs/constants live OUTSIDE the repeat loop so the
        # next iteration's first writes don't WAR-serialize against the tail
        # of the previous iteration
        xs8 = singles.tile([128, KB, B], f8)
        nc.sync.dma_start(out=xs8[:], in_=xs8_d)
        ones8 = singles.tile([128, 2, CBLK], f8e5)
        nc.vector.memset(ones8[:], 1.0)
        bias_sh = singles.tile([128, 1], f32)
        nc.vector.memset(bias_sh[:], -SH)

        ctx.enter_context(
            tc.For_i(0, repeat, 1, hint_engines=hint) if repeat > 1 else nullcontext()
        )

        sums = sump.tile([128, B], f32, tag="sums", name="sums")

        wsups = [None] * NSUP
        es_tiles = [None] * NSUP
        n_mms = sum((len(sup) + 1) // 2 for sup in supers)
        mm_idx = [0]  # running count for start/stop of the sum accum group

        def phase_dma(s):
            sup = supers[s]
            ns = len(sup)
            ws = wsp.tile(
                [128, SUP, KB, CBLK], f8, tag="ws", name=f"ws{s}",
                padded_shape=[128, SUP, KB, CBLK],
            )
            wsups[s] = ws
            nc.sync.dma_start(out=ws[:, :ns, :, :], in_=w8_d[:, sup[0] : sup[0] + ns, :, :])

        def phase_m(s):
            sup = supers[s]
            ns = len(sup)
            ws = wsups[s]
            es = esp.tile([128, SUP, B], f8e5, tag="es", name=f"es{s}")
            es_tiles[s] = es
            npairs = (ns + 1) // 2
            for pi in range(npairs):
                bis = [b for b in (2 * pi, 2 * pi + 1) if b < ns]
                np_ = len(bis)
                pm = pmp.tile([128, 2, B], f32, tag="pm", name=f"pm{s}_{pi}")
                for j, bi in enumerate(bis):
                    for kp in range(2):
                        nc.tensor.matmul(
                            pm[:, j, :],
                            lhsT=ws[:, bi, 2 * kp : 2 * kp + 2, :],
                            rhs=xs8[:, 2 * kp : 2 * kp + 2, :],
                            start=(kp == 0),
                            stop=(kp == 1),
                            perf_mode=DR,
                        )
                gpair = (sup[0] + 2 * pi) // 2
                if routes[gpair]:
                    # Schraudolph bit-trick exp on DVE: one fused mult+add
                    # with f32->u8 convert; u8 bits are the f8e5 encoding.
                    nc.vector.tensor_scalar(
                        out=es[:, 2 * pi : 2 * pi + np_, :].bitcast(u8),
                        in0=pm[:, :np_, :],
                        scalar1=A1,
                        scalar2=B1,
                        op0=mybir.AluOpType.mult,
                        op1=mybir.AluOpType.add,
                    )
                else:
                    nc.scalar.activation(
                        es[:, 2 * pi : 2 * pi + np_, :],
                        pm[:, :np_, :],
                        mybir.ActivationFunctionType.Exp,
                        bias=bias_sh[:],
                        scale=S / 256.0,
                    )

        def phase_sum(s):
            es = es_tiles[s]
            ns = len(supers[s])
            for pi in range((ns + 1) // 2):
                i = mm_idx[0]
                mm_idx[0] += 1
                nc.tensor.matmul(
                    sums[:, :],
                    lhsT=ones8[:],
                    rhs=es[:, 2 * pi : 2 * pi + 2, :],
                    start=(i == 0),
                    stop=(i == n_mms - 1),
                    perf_mode=DR,
                    skip_group_check=True,
                )

        # software-pipelined emission: DMA runs 1-3 supers ahead of mains,
        # sum-matmuls trail mains by 1 super so PE never waits on exp(s).
        phase_dma(0)
        for s in range(NSUP):
            if s + 1 < NSUP:
                phase_dma(s + 1)
            if s >= 2:
                phase_m(s - 2)
            if s >= 3:
                phase_sum(s - 3)
        phase_m(NSUP - 2)
        phase_sum(NSUP - 3)
        phase_m(NSUP - 1)
        phase_sum(NSUP - 2)
        phase_sum(NSUP - 1)

        s_sb = singles.tile([1, B], f32)
        nc.vector.tensor_copy(s_sb[:], sums[0:1, :])
        nc.sync.dma_start(
            out=s_out.rearrange("(one b) -> one b", one=1), in_=s_sb[:]
        )

    nc.compile()
    return nc


def _get_nc():
    if "nc" not in _CACHE:
        _CACHE["nc"] = _build_nc()
    return _CACHE["nc"]


def _prep_inputs(x, weights):
    """Host-side shard/layout prep: normalize rows of x AND w, scale+cast
    to fp8e4m3 (no on-device norm correction needed)."""
    x = np.asarray(x, dtype=np.float32)
    w = np.asarray(weights, dtype=np.float32)
    f8 = ml_dtypes.float8_e4m3

    xn = x / np.linalg.norm(x.astype(np.float64), axis=1, keepdims=True)
    # xs8[p, k, b] = 16*xn[b, k*128+p]
    xs8 = np.ascontiguousarray(
        (FSCALE * xn.T.astype(np.float32)).reshape(KB, 128, B).transpose(1, 0, 2)
    ).astype(f8)

    wn = w.astype(np.float64)
    wn = wn / np.linalg.norm(wn, axis=1, keepdims=True)
    wpad = np.zeros((CPAD, D), dtype=np.float32)
    wpad[:C] = wn.astype(np.float32)
    w8_maps = []
    for i in range(NCORES):
        shard = FSCALE * wpad[i * CSH : (i + 1) * CSH]  # [12800, 512]
        # w8[p, cb, k, c] = 16*shard[cb*128+c, k*128+p]
        arr = shard.reshape(NBLK, CBLK, KB, 128).transpose(3, 0, 2, 1)
        w8_maps.append(np.ascontiguousarray(arr).astype(f8))
    return xs8, w8_maps


def _in_maps(x, weights):
    xs8, w8_maps = _prep_inputs(x, weights)
    return [{"xs8": xs8, "w8": w8_maps[i]} for i in range(NCORES)]


def _run_on_device(in_maps, trace=False):
    from concourse.bass_utils import run_bass_kernel_spmd

    nc = _get_nc()
    res = run_bass_kernel_spmd(
        nc, in_maps, core_ids=list(range(NCORES)), trace=trace
    )
    _CACHE["last_results"] = res
    return [r["s_out"].astype(np.float64) for r in res.results]


def _pad_correction():
    """Exact contribution of the CPAD-C zero pad classes to the device sum.
    Act-path pads give e^-SH each; DVE-path pads give value(clip(rint(B1)))."""
    routes = _pair_routes()
    pad_bits = int(np.clip(np.rint(B1), 0, 255))
    dve_pad = float(np.uint8(pad_bits).view(ml_dtypes.float8_e5m2))
    pad_lo = C - (NCORES - 1) * CSH  # first pad class, local to core 7
    corr = 0.0
    for p in range(len(routes)):
        lo, hi = 256 * p, 256 * p + 256
        npad = max(0, hi - max(lo, pad_lo))
        corr += npad * (dve_pad if routes[p] else math.exp(-SH))
    return corr


def kernel(x, weights, targets, _trace=False):
    x = np.asarray(x)
    weights = np.asarray(weights)
    targets = np.asarray(targets).astype(np.int64)

    s_shards = _run_on_device(_in_maps(x, weights), trace=_trace)

    # ---- host combine (f64, ~0.5 MFLOP total) ----
    s_total = np.sum(s_shards, axis=0)  # [B]
    s_total = s_total - _pad_correction()

    xf = x.astype(np.float64)
    xn = xf / np.linalg.norm(xf, axis=1, keepdims=True)
    wtg = weights.astype(np.float64)[targets]  # [B, D] gathered target rows
    wtg = wtg / np.linalg.norm(wtg, axis=1, keepdims=True)
    cos_t = np.einsum("bd,bd->b", xn, wtg)

    sin_t = np.sqrt(np.clip(1.0 - cos_t * cos_t, 0.0, 1.0))
    phi = cos_t * COS_M - sin_t * SIN_M
    psi = np.where(cos_t > TH, phi, cos_t - MM)

    # swap the target term: remove exp(S*cos_t), add exp(S*psi)
    s_adj = s_total - np.exp(S * cos_t - SH) + np.exp(S * psi - SH)
    lse = SH + np.log(s_adj)
    loss = np.mean(lse - S * psi)
    return np.float32(loss)
